# revision 34
# baseline (speedup 1.0000x reference)
"""GQA causal self-attention on 8 trn2 NeuronCores (Bass/Tile, SPMD).

Sharding: 4 batches x 2 core types. Core c handles batch c//2; type A (c%2==0)
owns query rows [0:256)+[768:1024), type B rows [256:768) -- the two types do
equal causal work. Keys stay in natural order; the core's q rows arrive as a
separate host-sliced xTq input, so one program serves all cores; causality
comes from host-built 0/1 mask tiles (query chunk 0 masks key chunks 0-3,
query chunk 1 masks key chunks 4-7; all other processed tiles are full-keep
for both core types).

All matmuls run in float32r (tf32-class, full PE rate). Projections are
computed in transposed orientation: qT[dk, head, qrow], kT[dk, group, key],
v[key, group, dk(+ones)]. Attention uses scoresT = kT.T @ qT (keys on
partitions) so exp/mask/AV need no transposes; the ones-column of v makes the
attention-V matmul emit softmax denominators for free; normalization rides the
PSUM->SBUF copy; c_proj consumes yT directly and adds bias via a K=1 matmul.
"""
import sys

sys.path.insert(0, "/opt/trn_rl_repo")

import numpy as np

import concourse.bass as bass
import concourse.bacc as bacc
import concourse.mybir as mybir
import concourse.tile as tile
from concourse import bass_utils

dt = mybir.dt
AF = mybir.ActivationFunctionType

B, T, E = 4, 1024, 2048
H, G, DK = 32, 8, 64
REP = H // G  # 4
EC = E // 128  # 16 E-chunks
QR = 512  # q rows per core
NKC = T // 128  # 8 key chunks
KCS = (4, 8)  # key chunks processed for qc 0 (rows 0-255) / qc 1 (256-511)
N_CORES = 8

# q rows per core type: type 0 = A (quarters 0+3), type 1 = B (quarters 1+2)
def _qrows(ctype: int) -> np.ndarray:
    if ctype == 0:
        return np.concatenate([np.arange(0, 256), np.arange(768, 1024)])
    return np.arange(256, 768)


def _masks(ctype: int) -> np.ndarray:
    """[8, 128, 256] f32: idx 0-3 = qc0 kc0-3, idx 4-7 = qc1 kc4-7."""
    rows = _qrows(ctype)
    m = np.empty((8, 128, 256), np.float32)
    for i in range(8):
        qc, kc = (0, i) if i < 4 else (1, i)
        qpos = rows[qc * 256 : qc * 256 + 256]   # orig q rows of this chunk
        kpos = np.arange(kc * 128, kc * 128 + 128)
        m[i] = (qpos[None, :] >= kpos[:, None]).astype(np.float32)
    return m


def build():
    """Build + compile the single-core SPMD program. Returns nc."""
    nc = bacc.Bacc("TRN2", target_bir_lowering=False, debug=False,
                   num_devices=N_CORES)

    xT = nc.dram_tensor("xT", [E, T], dt.float32r, kind="ExternalInput").ap()
    xTq = nc.dram_tensor("xTq", [E, QR], dt.float32r,
                         kind="ExternalInput").ap()
    Wq = nc.dram_tensor("Wq", [E, E], dt.float32r, kind="ExternalInput").ap()
    Wkv = nc.dram_tensor("Wkv", [E, 2 * G * DK], dt.float32r,
                         kind="ExternalInput").ap()
    Wc = nc.dram_tensor("Wc", [E, E], dt.float32r, kind="ExternalInput").ap()
    bq = nc.dram_tensor("bq", [EC, 128], dt.float32r, kind="ExternalInput").ap()
    bk = nc.dram_tensor("bk", [G * DK // 128, 128], dt.float32r,
                        kind="ExternalInput").ap()
    bv = nc.dram_tensor("bv", [1, G * DK], dt.float32r,
                        kind="ExternalInput").ap()
    bc = nc.dram_tensor("bc", [1, E], dt.float32r, kind="ExternalInput").ap()
    masks = nc.dram_tensor("masks", [8, 128, 256], dt.float32r,
                           kind="ExternalInput").ap()

    y_out = nc.dram_tensor("y_out", [QR, E], dt.float32,
                           kind="ExternalOutput").ap()
    kT_out = nc.dram_tensor("kT_out", [G * DK, T], dt.float32,
                            kind="ExternalOutput").ap()
    v_out = nc.dram_tensor("v_out", [T, G * DK], dt.float32,
                           kind="ExternalOutput").ap()

    with tile.TileContext(nc) as tc:
        # ---------- persistent pools ----------
        with tc.tile_pool(name="persist", bufs=1) as pp:
            # head h lives at partition base 64*((h%8)//4) (same as its
            # group's kT half), column (h//8)*4 + (h%8)%4
            qT = pp.tile([128, H // 2, QR], dt.float32r)  # 4 MB
            # groups 0-3 on partitions 0-63, groups 4-7 on 64-127
            kT = pp.tile([128, G // 2, T], dt.float32r)   # 2 MB
            vsb = pp.tile([128, NKC, G, DK + 1], dt.float32r)  # ~2.1 MB
            bq_sb = pp.tile([128, EC], dt.float32r)
            bk_sb = pp.tile([128, G * DK // 128], dt.float32r)  # k-half of bkv
            bv_sb = pp.tile([1, G * DK], dt.float32r)     # v-half of bkv
            bc_sb = pp.tile([1, E], dt.float32r)
            ones_r = pp.tile([1, 128], dt.float32r)

            nc.sync.dma_start(bq_sb[:], bq.rearrange("c p -> p c"))
            nc.sync.dma_start(bk_sb[:], bk.rearrange("c p -> p c"))
            nc.sync.dma_start(bv_sb[:], bv)
            nc.sync.dma_start(bc_sb[:], bc)
            nc.gpsimd.memset(ones_r[:].bitcast(dt.float32), 1.0)

            # ================= phase 1: projections =================
            with tc.tile_pool(name="xk", bufs=1) as xp:
                xk = xp.tile([128, EC, T], dt.float32r)   # 8 MB
                xq = xp.tile([128, EC, QR], dt.float32r)  # 4 MB
                xTqv = xTq.rearrange("(e p) t -> p e t", p=128)
                xTv = xT.rearrange("(e p) t -> p e t", p=128)

                # ---- q projection: qT[dk, h, qrow] ----
                with tc.tile_pool(name="wq", bufs=8) as wqp, \
                     tc.tile_pool(name="qps", bufs=2, space="PSUM") as qpsp:
                    for ctg in range(4):          # groups of 4 col-tiles
                        q_ps = [qpsp.tile([128, QR], dt.float32, tag=f"q{s}",
                                          name=f"q_ps{s}") for s in range(4)]
                        for e in range(EC):
                            if ctg == 0:
                                nc.sync.dma_start(xq[:, e, :], xTqv[:, e, :])
                            elif e % 3 == ctg - 1:
                                nc.sync.dma_start(xk[:, e, :], xTv[:, e, :])
                            slab = wqp.tile([128, 512], dt.float32r, tag="wq")
                            nc.sync.dma_start(
                                slab[:],
                                Wq[bass.ts(e, 128), bass.ts(ctg, 512)])
                            for s in range(4):
                                nc.tensor.matmul(
                                    q_ps[s][:], slab[:, bass.ts(s, 128)],
                                    xq[:, e, :],
                                    start=(e == 0), stop=(e == EC - 1))

                        for s in range(4):
                            ct = ctg * 4 + s      # col tile = heads 2ct, 2ct+1
                            for half in range(2):
                                h = 2 * ct + half
                                pr = 64 * ((h % 8) // 4)
                                col = (h // 8) * 4 + (h % 8) % 4
                                nc.scalar.activation(
                                    qT[pr:pr + 64, col, :],
                                    q_ps[s][bass.ts(half, 64), :],
                                    AF.Identity,
                                    bias=bq_sb[bass.ts(half, 64), ct:ct + 1])

                # ---- k projection (transposed): kT[dk, g, key] ----
                with tc.tile_pool(name="wk", bufs=16) as wkp, \
                     tc.tile_pool(name="kps", bufs=3, space="PSUM") as kpsp:
                    for ct in range(4):           # col tiles over G*DK=512
                        k_ps = [kpsp.tile([128, 512], dt.float32, tag=f"k{i}",
                                          name=f"k_ps{i}") for i in range(2)]
                        for e in range(EC):
                            wt = wkp.tile([128, 128], dt.float32r, tag="wk")
                            nc.sync.dma_start(
                                wt[:], Wkv[bass.ts(e, 128), bass.ts(ct, 128)])
                            for half in range(2):
                                nc.tensor.matmul(
                                    k_ps[half][:], wt[:],
                                    xk[:, e, bass.ts(half, 512)],
                                    start=(e == 0), stop=(e == EC - 1))
                        for half in range(2):
                            for sub in range(2):
                                g = 2 * ct + sub
                                pr = 64 * (g // 4)
                                nc.scalar.activation(
                                    kT[pr:pr + 64, g % 4, bass.ts(half, 512)],
                                    k_ps[half][bass.ts(sub, 64), :],
                                    AF.Identity,
                                    bias=bk_sb[bass.ts(sub, 64), ct:ct + 1])

                # ---- v projection (natural): v[key, g, dk] + ones col ----
                # 8 psum accumulators (one per key tile); Wv streamed once
                with tc.tile_pool(name="wv", bufs=6) as wvp, \
                     tc.tile_pool(name="vps", bufs=1, space="PSUM") as vpsp:
                    v_ps = [vpsp.tile([128, 512], dt.float32, tag=f"v{kt}",
                                      name=f"v_ps{kt}") for kt in range(NKC)]
                    for e in range(EC):
                        wvt = wvp.tile([128, 512], dt.float32r, tag="wv")
                        nc.sync.dma_start(
                            wvt[:],
                            Wkv[bass.ts(e, 128), G * DK:2 * G * DK])
                        for kt in range(NKC):
                            nc.tensor.matmul(
                                v_ps[kt][:], xk[:, e, bass.ts(kt, 128)],
                                wvt[:], start=(e == 0), stop=False)
                    for kt in range(NKC):
                        # + bias row (K=1): ones[1,128].T @ bv[1,512]
                        nc.tensor.matmul(v_ps[kt][:], ones_r[:], bv_sb[:],
                                         start=False, stop=True)
                        ceng = nc.scalar if kt % 2 == 0 else nc.vector
                        if kt % 2 == 0:
                            nc.scalar.copy(vsb[:, kt, :, 0:DK],
                                           v_ps[kt][:].rearrange(
                                               "p (g d) -> p g d", g=G))
                        else:
                            nc.vector.tensor_copy(
                                vsb[:, kt, :, 0:DK],
                                v_ps[kt][:].rearrange("p (g d) -> p g d", g=G))
                        for g in range(G):
                            nc.gpsimd.memset(
                                vsb[:, kt, g, DK:DK + 1].bitcast(dt.float32), 1.0)


            # ================= phases 2+3 =================
            with tc.tile_pool(name="yT", bufs=1) as ytp:
                yT = ytp.tile([128, EC, QR], dt.float32r)     # 4 MB
                msk = ytp.tile([128, 8, 256], dt.float32r)    # 1 MB
                nc.sync.dma_start(msk[:], masks.rearrange("m p q -> p m q"))
                with tc.tile_pool(name="ex", bufs=6) as exp_, \
                     tc.tile_pool(name="bcst", bufs=4) as bcp, \
                     tc.tile_pool(name="wc", bufs=16) as wcp:
                    # ---- attention ----
                    with tc.tile_pool(name="scps", bufs=3, space="PSUM") as scp, \
                         tc.tile_pool(name="avps", bufs=2, space="PSUM") as avp:
                        for qc in range(2):
                            nkc = KCS[qc]
                            for g in range(G):
                                for hp in range(2):
                                    h0 = g + 16 * hp  # heads h0, h0+8 (grp g)
                                    av = avp.tile([65, 512], dt.float32,
                                                  tag="av")
                                    base = 64 * (g // 4)
                                    rhs = qT[base:base + 64, :, :] \
                                        .rearrange("p (j c) q -> p j c q",
                                                   j=4)[:, 2 * hp:2 * hp + 2,
                                                        g % 4, bass.ts(qc, 256)]
                                    exs = []
                                    for kp2 in range(nkc // 2):
                                        # two key chunks share one psum/exp op
                                        sc2 = scp.tile([128, 2, 512],
                                                       dt.float32, tag="sc",
                                                       name="sc2")
                                        for j in range(2):
                                            kci = 2 * kp2 + j
                                            lhsT = kT[base:base + 64, g % 4,
                                                      bass.ts(kci, 128)]
                                            nc.tensor.matmul(
                                                sc2[:, j, :], lhsT, rhs,
                                                start=True, stop=True)
                                        ex2 = exp_.tile([128, 2, 2, 256],
                                                        dt.float32r, tag="ex",
                                                        name=f"ex{kp2}",
                                                        bufs=8)
                                        nc.scalar.activation(
                                            ex2[:].rearrange(
                                                "p k h q -> p (k h q)"),
                                            sc2[:].rearrange(
                                                "p k n -> p (k n)"),
                                            AF.Exp,
                                            scale=float(1.0 / np.sqrt(DK)))
                                        masked = (qc == 0) or 2 * kp2 >= 4
                                        if masked:
                                            mb = msk[:, 2 * kp2:2 * kp2 + 2,
                                                     None, :] \
                                                .broadcast_to((128, 2, 2, 256))
                                            nc.vector.tensor_mul(ex2[:],
                                                                 ex2[:], mb)
                                        exs.append(ex2)
                                    for kci in range(nkc):
                                        ex2 = exs[kci // 2]
                                        nc.tensor.matmul(
                                            av[:], vsb[:, kci, g, :],
                                            ex2[:, kci % 2, :, :].rearrange(
                                                "p h q -> p (h q)"),
                                            start=(kci == 0),
                                            stop=(kci == nkc - 1))
                                    # normalize: yT_h = av[0:64] * (1/av[64])
                                    rc = bcp.tile([1, 512], dt.float32,
                                                  tag="rc")
                                    nc.vector.reciprocal(rc[:], av[64:65, :])
                                    bcst = bcp.tile([64, 512], dt.float32,
                                                    tag="bcst")
                                    nc.gpsimd.partition_broadcast(bcst[:],
                                                                  rc[:])
                                    p0 = 64 * (g % 2)
                                    yTv = yT[p0:p0 + 64, :, :].rearrange(
                                        "p (j c) q -> p j c q", j=4)
                                    nc.vector.tensor_mul(
                                        yTv[:, 2 * hp:2 * hp + 2, g // 2,
                                            bass.ts(qc, 256)],
                                        av[0:64, :].rearrange(
                                            "p (h q) -> p h q", h=2),
                                        bcst[:, :].rearrange(
                                            "p (h q) -> p h q", h=2))

                    # ---- kv outputs (independent; emitted late) ----
                    for g in range(G):
                        pr = 64 * (g // 4)
                        nc.sync.dma_start(
                            kT_out[bass.ts(g, 64), :],
                            kT[pr:pr + 64, g % 4, :].bitcast(dt.float32))
                    for kt in range(NKC):
                        nc.sync.dma_start(
                            v_out[bass.ts(kt, 128), :].rearrange(
                                "p (g d) -> p g d", g=G),
                            vsb[:, kt, :, 0:DK].bitcast(dt.float32))

                    # ---- c_proj: y_out[qr, oc] = yT.T @ Wc + bc ----
                    with tc.tile_pool(name="ops", bufs=2,
                                      space="PSUM") as opp:
                        for oc in range(4):
                            o_ps = [opp.tile([128, 512], dt.float32, tag=f"o{qt}",
                                             name=f"o_ps{qt}")
                                    for qt in range(4)]
                            for e in range(EC):
                                slab = wcp.tile([128, 512], dt.float32r,
                                                tag="wc")
                                nc.sync.dma_start(
                                    slab[:],
                                    Wc[bass.ts(e, 128), bass.ts(oc, 512)])
                                for qt in range(4):
                                    nc.tensor.matmul(
                                        o_ps[qt][:],
                                        yT[:, e, bass.ts(qt, 128)],
                                        slab[:], start=(e == 0), stop=False)
                            for qt in range(4):
                                nc.tensor.matmul(o_ps[qt][:], ones_r[:],
                                                 bc_sb[:, bass.ts(oc, 512)],
                                                 start=False, stop=True)
                                ot = exp_.tile([128, 512], dt.float32,
                                               tag="ot")
                                nc.scalar.copy(ot[:], o_ps[qt][:])
                                nc.sync.dma_start(
                                    y_out[bass.ts(qt, 128),
                                          bass.ts(oc, 512)],
                                    ot[:])

    nc.compile()
    return nc


_NC = None


def _get_nc():
    global _NC
    if _NC is None:
        _NC = build()
    return _NC


def make_in_maps(x, Wq, bq, Wkv, bkv, Wc, bc):
    x = np.asarray(x, np.float32)
    shared = {
        "Wq": np.asarray(Wq, np.float32),
        "Wkv": np.asarray(Wkv, np.float32),
        "Wc": np.asarray(Wc, np.float32),
        "bq": np.asarray(bq, np.float32).reshape(EC, 128),
        "bk": np.asarray(bkv, np.float32)[:G * DK].reshape(G * DK // 128, 128),
        "bv": np.asarray(bkv, np.float32)[G * DK:].reshape(1, G * DK),
        "bc": np.asarray(bc, np.float32).reshape(1, E),
    }
    masks_t = [_masks(0), _masks(1)]
    qrows_t = [_qrows(0), _qrows(1)]
    in_maps = []
    for c in range(N_CORES):
        b, t = c // 2, c % 2
        xTb = np.ascontiguousarray(x[b].T)
        m = dict(shared)
        m["xT"] = xTb
        m["xTq"] = np.ascontiguousarray(xTb[:, qrows_t[t]])
        m["masks"] = masks_t[t]
        in_maps.append(m)
    return in_maps


def assemble(results):
    """results: list of 8 per-core dicts -> (y, k, v) full outputs."""
    qrows_t = [_qrows(0), _qrows(1)]
    y = np.empty((B, T, E), np.float32)
    k = np.empty((B, G, T, DK), np.float32)
    v = np.empty((B, G, T, DK), np.float32)
    for c in range(N_CORES):
        b, t = c // 2, c % 2
        y[b, qrows_t[t]] = results[c]["y_out"]
        if t == 0:
            k[b] = results[c]["kT_out"].reshape(G, DK, T).transpose(0, 2, 1)
            v[b] = results[c]["v_out"].reshape(T, G, DK).transpose(1, 0, 2)
    k_full = np.tile(k, (1, REP, 1, 1))
    v_full = np.tile(v, (1, REP, 1, 1))
    return y, k_full, v_full


def kernel(**inputs):
    nc = _get_nc()
    in_maps = make_in_maps(**inputs)
    try:
        res = bass_utils.run_bass_kernel_spmd(nc, in_maps,
                                              core_ids=list(range(N_CORES)))
    except Exception:
        # transient device errors (e.g. NRT_EXEC_UNIT_UNRECOVERABLE) have
        # been observed to clear on retry
        res = bass_utils.run_bass_kernel_spmd(nc, in_maps,
                                              core_ids=list(range(N_CORES)))
    return assemble(res.results)


# revision 37
# speedup vs baseline: 1.7902x; 1.7902x over previous
"""GQA causal self-attention on 8 trn2 NeuronCores (Bass/Tile, SPMD).

Sharding: 4 batches x 2 core types. Core c handles batch c//2; type A (c%2==0)
owns query rows [0:256)+[768:1024), type B rows [256:768) -- the two types do
equal causal work. Keys stay in natural order; the core's q rows arrive as a
separate host-sliced xTq input, so one program serves all cores; causality
comes from host-built 0/1 mask tiles (query chunk 0 masks key chunks 0-3,
query chunk 1 masks key chunks 4-7; all other processed tiles are full-keep
for both core types).

All matmuls run in float32r (tf32-class, full PE rate). Projections are
computed in transposed orientation: qT[dk, head, qrow], kT[dk, group, key],
v[key, group, dk(+ones)]. Attention uses scoresT = kT.T @ qT (keys on
partitions) so exp/mask/AV need no transposes; the ones-column of v makes the
attention-V matmul emit softmax denominators for free; normalization rides the
PSUM->SBUF copy; c_proj consumes yT directly and adds bias via a K=1 matmul.
"""
import sys

sys.path.insert(0, "/opt/trn_rl_repo")

import numpy as np

import concourse.bass as bass
import concourse.bacc as bacc
import concourse.mybir as mybir
import concourse.tile as tile
from concourse import bass_utils

dt = mybir.dt
AF = mybir.ActivationFunctionType

B, T, E = 4, 1024, 2048
H, G, DK = 32, 8, 64
REP = H // G  # 4
EC = E // 128  # 16 E-chunks
QR = 512  # q rows per core
NKC = T // 128  # 8 key chunks
KCS = (4, 8)  # key chunks processed for qc 0 (rows 0-255) / qc 1 (256-511)
N_CORES = 8

# q rows per core type: type 0 = A (quarters 0+3), type 1 = B (quarters 1+2)
def _qrows(ctype: int) -> np.ndarray:
    if ctype == 0:
        return np.concatenate([np.arange(0, 256), np.arange(768, 1024)])
    return np.arange(256, 768)


def _masks(ctype: int) -> np.ndarray:
    """[8, 128, 256] f32: idx 0-3 = qc0 kc0-3, idx 4-7 = qc1 kc4-7."""
    rows = _qrows(ctype)
    m = np.empty((8, 128, 256), np.float32)
    for i in range(8):
        qc, kc = (0, i) if i < 4 else (1, i)
        qpos = rows[qc * 256 : qc * 256 + 256]   # orig q rows of this chunk
        kpos = np.arange(kc * 128, kc * 128 + 128)
        m[i] = (qpos[None, :] >= kpos[:, None]).astype(np.float32)
    return m


def build():
    """Build + compile the single-core SPMD program. Returns nc."""
    nc = bacc.Bacc("TRN2", target_bir_lowering=False, debug=False,
                   num_devices=N_CORES)

    xT = nc.dram_tensor("xT", [E, T], dt.float32r, kind="ExternalInput").ap()
    xTq = nc.dram_tensor("xTq", [E, QR], dt.float32r,
                         kind="ExternalInput").ap()
    Wq = nc.dram_tensor("Wq", [E, E], dt.float32r, kind="ExternalInput").ap()
    Wkv = nc.dram_tensor("Wkv", [E, 2 * G * DK], dt.float32r,
                         kind="ExternalInput").ap()
    Wc = nc.dram_tensor("Wc", [E, E], dt.float32r, kind="ExternalInput").ap()
    bq = nc.dram_tensor("bq", [EC, 128], dt.float32r, kind="ExternalInput").ap()
    bk = nc.dram_tensor("bk", [G * DK // 128, 128], dt.float32r,
                        kind="ExternalInput").ap()
    bv = nc.dram_tensor("bv", [1, G * DK], dt.float32r,
                        kind="ExternalInput").ap()
    bc = nc.dram_tensor("bc", [1, E], dt.float32r, kind="ExternalInput").ap()
    masks = nc.dram_tensor("masks", [8, 128, 256], dt.float32r,
                           kind="ExternalInput").ap()

    y_out = nc.dram_tensor("y_out", [QR, E], dt.float32,
                           kind="ExternalOutput").ap()
    kT_out = nc.dram_tensor("kT_out", [G * DK, T], dt.float32,
                            kind="ExternalOutput").ap()
    v_out = nc.dram_tensor("v_out", [T, G * DK], dt.float32,
                           kind="ExternalOutput").ap()

    with tile.TileContext(nc) as tc:
        # ---------- persistent pools ----------
        with tc.tile_pool(name="persist", bufs=1) as pp:
            # head h lives at partition base 64*((h%8)//4) (same as its
            # group's kT half), column (h//8)*4 + (h%8)%4
            qT = pp.tile([128, H // 2, QR], dt.float32r)  # 4 MB
            # groups 0-3 on partitions 0-63, groups 4-7 on 64-127
            kT = pp.tile([128, G // 2, T], dt.float32r)   # 2 MB
            vsb = pp.tile([128, NKC, G, DK + 1], dt.float32r)  # ~2.1 MB
            bq_sb = pp.tile([128, EC], dt.float32r)
            bk_sb = pp.tile([128, G * DK // 128], dt.float32r)  # k-half of bkv
            bv_sb = pp.tile([1, G * DK], dt.float32r)     # v-half of bkv
            bc_sb = pp.tile([1, E], dt.float32r)
            ones_r = pp.tile([1, 128], dt.float32r)

            nc.sync.dma_start(bq_sb[:], bq.rearrange("c p -> p c"))
            nc.sync.dma_start(bk_sb[:], bk.rearrange("c p -> p c"))
            nc.sync.dma_start(bv_sb[:], bv)
            nc.sync.dma_start(bc_sb[:], bc)
            nc.gpsimd.memset(ones_r[:].bitcast(dt.float32), 1.0)

            # ================= phase 1: projections =================
            with tc.tile_pool(name="xk", bufs=1) as xp:
                xk = xp.tile([128, EC, T], dt.float32r)   # 8 MB
                xq = xp.tile([128, EC, QR], dt.float32r)  # 4 MB
                xTqv = xTq.rearrange("(e p) t -> p e t", p=128)
                xTv = xT.rearrange("(e p) t -> p e t", p=128)

                # ---- q projection: qT[dk, h, qrow] ----
                with tc.tile_pool(name="wq", bufs=8) as wqp, \
                     tc.tile_pool(name="qps", bufs=2, space="PSUM") as qpsp:
                    for ctg in range(4):          # groups of 4 col-tiles
                        q_ps = [qpsp.tile([128, QR], dt.float32, tag=f"q{s}",
                                          name=f"q_ps{s}") for s in range(4)]
                        for e in range(EC):
                            if ctg == 0:
                                nc.sync.dma_start(xq[:, e, :], xTqv[:, e, :])
                            elif e % 3 == ctg - 1:
                                nc.sync.dma_start(xk[:, e, :], xTv[:, e, :])
                            slab = wqp.tile([128, 512], dt.float32r, tag="wq")
                            nc.sync.dma_start(
                                slab[:],
                                Wq[bass.ts(e, 128), bass.ts(ctg, 512)])
                            for s in range(4):
                                nc.tensor.matmul(
                                    q_ps[s][:], slab[:, bass.ts(s, 128)],
                                    xq[:, e, :],
                                    start=(e == 0), stop=(e == EC - 1))

                        for s in range(4):
                            ct = ctg * 4 + s      # col tile = heads 2ct, 2ct+1
                            for half in range(2):
                                h = 2 * ct + half
                                pr = 64 * ((h % 8) // 4)
                                col = (h // 8) * 4 + (h % 8) % 4
                                nc.scalar.activation(
                                    qT[pr:pr + 64, col, :],
                                    q_ps[s][bass.ts(half, 64), :],
                                    AF.Identity,
                                    bias=bq_sb[bass.ts(half, 64), ct:ct + 1])

                # ---- k projection (transposed): kT[dk, g, key] ----
                with tc.tile_pool(name="wk", bufs=16) as wkp, \
                     tc.tile_pool(name="kps", bufs=3, space="PSUM") as kpsp:
                    for ct in range(4):           # col tiles over G*DK=512
                        k_ps = [kpsp.tile([128, 512], dt.float32, tag=f"k{i}",
                                          name=f"k_ps{i}") for i in range(2)]
                        for e in range(EC):
                            wt = wkp.tile([128, 128], dt.float32r, tag="wk")
                            nc.sync.dma_start(
                                wt[:], Wkv[bass.ts(e, 128), bass.ts(ct, 128)])
                            for half in range(2):
                                nc.tensor.matmul(
                                    k_ps[half][:], wt[:],
                                    xk[:, e, bass.ts(half, 512)],
                                    start=(e == 0), stop=(e == EC - 1))
                        for half in range(2):
                            for sub in range(2):
                                g = 2 * ct + sub
                                pr = 64 * (g // 4)
                                nc.scalar.activation(
                                    kT[pr:pr + 64, g % 4, bass.ts(half, 512)],
                                    k_ps[half][bass.ts(sub, 64), :],
                                    AF.Identity,
                                    bias=bk_sb[bass.ts(sub, 64), ct:ct + 1])

                # ---- v projection (natural): v[key, g, dk] + ones col ----
                # 8 psum accumulators (one per key tile); Wv streamed once
                with tc.tile_pool(name="wv", bufs=6) as wvp, \
                     tc.tile_pool(name="vps", bufs=1, space="PSUM") as vpsp:
                    v_ps = [vpsp.tile([128, 512], dt.float32, tag=f"v{kt}",
                                      name=f"v_ps{kt}") for kt in range(NKC)]
                    for e in range(EC):
                        wvt = wvp.tile([128, 512], dt.float32r, tag="wv")
                        nc.sync.dma_start(
                            wvt[:],
                            Wkv[bass.ts(e, 128), G * DK:2 * G * DK])
                        for kt in range(NKC):
                            nc.tensor.matmul(
                                v_ps[kt][:], xk[:, e, bass.ts(kt, 128)],
                                wvt[:], start=(e == 0), stop=False)
                    for kt in range(NKC):
                        # + bias row (K=1): ones[1,128].T @ bv[1,512]
                        nc.tensor.matmul(v_ps[kt][:], ones_r[:], bv_sb[:],
                                         start=False, stop=True)
                        ceng = nc.scalar if kt % 2 == 0 else nc.vector
                        if kt % 2 == 0:
                            nc.scalar.copy(vsb[:, kt, :, 0:DK],
                                           v_ps[kt][:].rearrange(
                                               "p (g d) -> p g d", g=G))
                        else:
                            nc.vector.tensor_copy(
                                vsb[:, kt, :, 0:DK],
                                v_ps[kt][:].rearrange("p (g d) -> p g d", g=G))
                        for g in range(G):
                            nc.gpsimd.memset(
                                vsb[:, kt, g, DK:DK + 1].bitcast(dt.float32), 1.0)


            # ================= phases 2+3 =================
            with tc.tile_pool(name="yT", bufs=1) as ytp:
                yT = ytp.tile([128, EC, QR], dt.float32r)     # 4 MB
                msk = ytp.tile([128, 8, 256], dt.float32r)    # 1 MB
                nc.sync.dma_start(msk[:], masks.rearrange("m p q -> p m q"))
                with tc.tile_pool(name="ex", bufs=6) as exp_, \
                     tc.tile_pool(name="bcst", bufs=4) as bcp, \
                     tc.tile_pool(name="wc", bufs=16) as wcp:
                    # ---- attention ----
                    with tc.tile_pool(name="scps", bufs=3, space="PSUM") as scp, \
                         tc.tile_pool(name="avps", bufs=2, space="PSUM") as avp:
                        for g in range(G):
                            for hp in range(2):
                                # qc1 (ACT-heavy) first, then qc0 (DVE-heavy):
                                # interleaving balances both engines
                                for qc in (1, 0):
                                    nkc = KCS[qc]
                                    h0 = g + 16 * hp  # heads h0, h0+8 (grp g)
                                    av = avp.tile([65, 512], dt.float32,
                                                  tag="av")
                                    base = 64 * (g // 4)
                                    rhs = qT[base:base + 64, :, :] \
                                        .rearrange("p (j c) q -> p j c q",
                                                   j=4)[:, 2 * hp:2 * hp + 2,
                                                        g % 4, bass.ts(qc, 256)]
                                    exs = []
                                    for kp2 in range(nkc // 2):
                                        # two key chunks share one psum/exp op
                                        sc2 = scp.tile([128, 2, 512],
                                                       dt.float32, tag="sc",
                                                       name="sc2")
                                        for j in range(2):
                                            kci = 2 * kp2 + j
                                            lhsT = kT[base:base + 64, g % 4,
                                                      bass.ts(kci, 128)]
                                            nc.tensor.matmul(
                                                sc2[:, j, :], lhsT, rhs,
                                                start=True, stop=True)
                                        ex2 = exp_.tile([128, 2, 2, 256],
                                                        dt.float32r, tag="ex",
                                                        name=f"ex{kp2}",
                                                        bufs=8)
                                        nc.scalar.activation(
                                            ex2[:].rearrange(
                                                "p k h q -> p (k h q)"),
                                            sc2[:].rearrange(
                                                "p k n -> p (k n)"),
                                            AF.Exp,
                                            scale=float(1.0 / np.sqrt(DK)))
                                        masked = (qc == 0) or 2 * kp2 >= 4
                                        if masked:
                                            mb = msk[:, 2 * kp2:2 * kp2 + 2,
                                                     None, :] \
                                                .broadcast_to((128, 2, 2, 256))
                                            nc.vector.tensor_mul(ex2[:],
                                                                 ex2[:], mb)
                                        exs.append(ex2)
                                    for kci in range(nkc):
                                        ex2 = exs[kci // 2]
                                        nc.tensor.matmul(
                                            av[:], vsb[:, kci, g, :],
                                            ex2[:, kci % 2, :, :].rearrange(
                                                "p h q -> p (h q)"),
                                            start=(kci == 0),
                                            stop=(kci == nkc - 1))
                                    # normalize: yT_h = av[0:64] * (1/av[64])
                                    rc = bcp.tile([1, 512], dt.float32,
                                                  tag="rc")
                                    nc.vector.reciprocal(rc[:], av[64:65, :])
                                    bcst = bcp.tile([64, 512], dt.float32,
                                                    tag="bcst")
                                    nc.gpsimd.partition_broadcast(bcst[:],
                                                                  rc[:])
                                    p0 = 64 * (g % 2)
                                    yTv = yT[p0:p0 + 64, :, :].rearrange(
                                        "p (j c) q -> p j c q", j=4)
                                    nc.vector.tensor_mul(
                                        yTv[:, 2 * hp:2 * hp + 2, g // 2,
                                            bass.ts(qc, 256)],
                                        av[0:64, :].rearrange(
                                            "p (h q) -> p h q", h=2),
                                        bcst[:, :].rearrange(
                                            "p (h q) -> p h q", h=2))

                    # ---- kv outputs (independent; emitted late) ----
                    for g in range(G):
                        pr = 64 * (g // 4)
                        nc.sync.dma_start(
                            kT_out[bass.ts(g, 64), :],
                            kT[pr:pr + 64, g % 4, :].bitcast(dt.float32))
                    for kt in range(NKC):
                        nc.sync.dma_start(
                            v_out[bass.ts(kt, 128), :].rearrange(
                                "p (g d) -> p g d", g=G),
                            vsb[:, kt, :, 0:DK].bitcast(dt.float32))

                    # ---- c_proj: y_out[qr, oc] = yT.T @ Wc + bc ----
                    with tc.tile_pool(name="ops", bufs=2,
                                      space="PSUM") as opp:
                        for oc in range(4):
                            o_ps = [opp.tile([128, 512], dt.float32, tag=f"o{qt}",
                                             name=f"o_ps{qt}")
                                    for qt in range(4)]
                            for e in range(EC):
                                slab = wcp.tile([128, 512], dt.float32r,
                                                tag="wc")
                                nc.sync.dma_start(
                                    slab[:],
                                    Wc[bass.ts(e, 128), bass.ts(oc, 512)])
                                for qt in range(4):
                                    nc.tensor.matmul(
                                        o_ps[qt][:],
                                        yT[:, e, bass.ts(qt, 128)],
                                        slab[:], start=(e == 0), stop=False)
                            for qt in range(4):
                                nc.tensor.matmul(o_ps[qt][:], ones_r[:],
                                                 bc_sb[:, bass.ts(oc, 512)],
                                                 start=False, stop=True)
                                ot = exp_.tile([128, 512], dt.float32,
                                               tag="ot")
                                nc.scalar.copy(ot[:], o_ps[qt][:])
                                nc.sync.dma_start(
                                    y_out[bass.ts(qt, 128),
                                          bass.ts(oc, 512)],
                                    ot[:])

    nc.compile()
    return nc


_NC = None


def _get_nc():
    global _NC
    if _NC is None:
        _NC = build()
    return _NC


def make_in_maps(x, Wq, bq, Wkv, bkv, Wc, bc):
    x = np.asarray(x, np.float32)
    shared = {
        "Wq": np.asarray(Wq, np.float32),
        "Wkv": np.asarray(Wkv, np.float32),
        "Wc": np.asarray(Wc, np.float32),
        "bq": np.asarray(bq, np.float32).reshape(EC, 128),
        "bk": np.asarray(bkv, np.float32)[:G * DK].reshape(G * DK // 128, 128),
        "bv": np.asarray(bkv, np.float32)[G * DK:].reshape(1, G * DK),
        "bc": np.asarray(bc, np.float32).reshape(1, E),
    }
    masks_t = [_masks(0), _masks(1)]
    qrows_t = [_qrows(0), _qrows(1)]
    in_maps = []
    for c in range(N_CORES):
        b, t = c // 2, c % 2
        xTb = np.ascontiguousarray(x[b].T)
        m = dict(shared)
        m["xT"] = xTb
        m["xTq"] = np.ascontiguousarray(xTb[:, qrows_t[t]])
        m["masks"] = masks_t[t]
        in_maps.append(m)
    return in_maps


def assemble(results):
    """results: list of 8 per-core dicts -> (y, k, v) full outputs."""
    qrows_t = [_qrows(0), _qrows(1)]
    y = np.empty((B, T, E), np.float32)
    k = np.empty((B, G, T, DK), np.float32)
    v = np.empty((B, G, T, DK), np.float32)
    for c in range(N_CORES):
        b, t = c // 2, c % 2
        y[b, qrows_t[t]] = results[c]["y_out"]
        if t == 0:
            k[b] = results[c]["kT_out"].reshape(G, DK, T).transpose(0, 2, 1)
            v[b] = results[c]["v_out"].reshape(T, G, DK).transpose(1, 0, 2)
    k_full = np.tile(k, (1, REP, 1, 1))
    v_full = np.tile(v, (1, REP, 1, 1))
    return y, k_full, v_full


def kernel(**inputs):
    nc = _get_nc()
    in_maps = make_in_maps(**inputs)
    try:
        res = bass_utils.run_bass_kernel_spmd(nc, in_maps,
                                              core_ids=list(range(N_CORES)))
    except Exception:
        # transient device errors (e.g. NRT_EXEC_UNIT_UNRECOVERABLE) have
        # been observed to clear on retry
        res = bass_utils.run_bass_kernel_spmd(nc, in_maps,
                                              core_ids=list(range(N_CORES)))
    return assemble(res.results)


# revision 40
# speedup vs baseline: 4.2433x; 2.3702x over previous
"""GQA causal self-attention on 8 trn2 NeuronCores (Bass/Tile, SPMD).

Sharding: 4 batches x 2 core types. Core c handles batch c//2; type A (c%2==0)
owns query rows [0:256)+[768:1024), type B rows [256:768) -- the two types do
equal causal work. Keys stay in natural order; the core's q rows arrive as a
separate host-sliced xTq input, so one program serves all cores; causality
comes from host-built 0/1 mask tiles (query chunk 0 masks key chunks 0-3,
query chunk 1 masks key chunks 4-7; all other processed tiles are full-keep
for both core types).

All matmuls run in float32r (tf32-class, full PE rate). Projections are
computed in transposed orientation: qT[dk, head, qrow], kT[dk, group, key],
v[key, group, dk(+ones)]. Attention uses scoresT = kT.T @ qT (keys on
partitions) so exp/mask/AV need no transposes; the ones-column of v makes the
attention-V matmul emit softmax denominators for free; normalization rides the
PSUM->SBUF copy; c_proj consumes yT directly and adds bias via a K=1 matmul.
"""
import sys

sys.path.insert(0, "/opt/trn_rl_repo")

import numpy as np

import concourse.bass as bass
import concourse.bacc as bacc
import concourse.mybir as mybir
import concourse.tile as tile
from concourse import bass_utils

dt = mybir.dt
AF = mybir.ActivationFunctionType

B, T, E = 4, 1024, 2048
H, G, DK = 32, 8, 64
REP = H // G  # 4
EC = E // 128  # 16 E-chunks
QR = 512  # q rows per core
NKC = T // 128  # 8 key chunks
KCS = (4, 8)  # key chunks processed for qc 0 (rows 0-255) / qc 1 (256-511)
N_CORES = 8

# q rows per core type: type 0 = A (quarters 0+3), type 1 = B (quarters 1+2)
def _qrows(ctype: int) -> np.ndarray:
    if ctype == 0:
        return np.concatenate([np.arange(0, 256), np.arange(768, 1024)])
    return np.arange(256, 768)


def _masks(ctype: int) -> np.ndarray:
    """[8, 128, 256] f32: idx 0-3 = qc0 kc0-3, idx 4-7 = qc1 kc4-7."""
    rows = _qrows(ctype)
    m = np.empty((8, 128, 256), np.float32)
    for i in range(8):
        qc, kc = (0, i) if i < 4 else (1, i)
        qpos = rows[qc * 256 : qc * 256 + 256]   # orig q rows of this chunk
        kpos = np.arange(kc * 128, kc * 128 + 128)
        m[i] = (qpos[None, :] >= kpos[:, None]).astype(np.float32)
    return m


def build():
    """Build + compile the single-core SPMD program. Returns nc."""
    nc = bacc.Bacc("TRN2", target_bir_lowering=False, debug=False,
                   num_devices=N_CORES)

    xT = nc.dram_tensor("xT", [E, T], dt.float32r, kind="ExternalInput").ap()
    xTq = nc.dram_tensor("xTq", [E, QR], dt.float32r,
                         kind="ExternalInput").ap()
    Wq = nc.dram_tensor("Wq", [E, E], dt.float32r, kind="ExternalInput").ap()
    Wkv = nc.dram_tensor("Wkv", [E, 2 * G * DK], dt.float32r,
                         kind="ExternalInput").ap()
    Wc = nc.dram_tensor("Wc", [E, E], dt.float32r, kind="ExternalInput").ap()
    bq = nc.dram_tensor("bq", [EC, 128], dt.float32r, kind="ExternalInput").ap()
    bk = nc.dram_tensor("bk", [G * DK // 128, 128], dt.float32r,
                        kind="ExternalInput").ap()
    bv = nc.dram_tensor("bv", [1, G * DK], dt.float32r,
                        kind="ExternalInput").ap()
    bc = nc.dram_tensor("bc", [1, E], dt.float32r, kind="ExternalInput").ap()
    masks = nc.dram_tensor("masks", [8, 128, 256], dt.float32r,
                           kind="ExternalInput").ap()

    y_out = nc.dram_tensor("y_out", [QR, E], dt.float32,
                           kind="ExternalOutput").ap()
    kT_out = nc.dram_tensor("kT_out", [G * DK, T], dt.float32,
                            kind="ExternalOutput").ap()
    v_out = nc.dram_tensor("v_out", [T, G * DK], dt.float32,
                           kind="ExternalOutput").ap()

    with tile.TileContext(nc) as tc:
        # ---------- persistent pools ----------
        with tc.tile_pool(name="persist", bufs=1) as pp:
            # head h lives at partition base 64*((h%8)//4) (same as its
            # group's kT half), column (h//8)*4 + (h%8)%4
            qT = pp.tile([128, H // 2, QR], dt.float32r)  # 4 MB
            # groups 0-3 on partitions 0-63, groups 4-7 on 64-127
            kT = pp.tile([128, G // 2, T], dt.float32r)   # 2 MB
            vsb = pp.tile([128, NKC, G, DK + 1], dt.float32r)  # ~2.1 MB
            bq_sb = pp.tile([128, EC], dt.float32r)
            bk_sb = pp.tile([128, G * DK // 128], dt.float32r)  # k-half of bkv
            bv_sb = pp.tile([1, G * DK], dt.float32r)     # v-half of bkv
            bc_sb = pp.tile([1, E], dt.float32r)
            ones_r = pp.tile([1, 128], dt.float32r)

            nc.sync.dma_start(bq_sb[:], bq.rearrange("c p -> p c"))
            nc.sync.dma_start(bk_sb[:], bk.rearrange("c p -> p c"))
            nc.sync.dma_start(bv_sb[:], bv)
            nc.sync.dma_start(bc_sb[:], bc)
            nc.gpsimd.memset(ones_r[:].bitcast(dt.float32), 1.0)

            # ================= phase 1: projections =================
            with tc.tile_pool(name="xk", bufs=1) as xp:
                xk = xp.tile([128, EC, T], dt.float32r)   # 8 MB
                xq = xp.tile([128, EC, QR], dt.float32r)  # 4 MB
                xTqv = xTq.rearrange("(e p) t -> p e t", p=128)
                xTv = xT.rearrange("(e p) t -> p e t", p=128)

                # ---- q projection: qT[dk, h, qrow] ----
                with tc.tile_pool(name="wq", bufs=8) as wqp, \
                     tc.tile_pool(name="qps", bufs=2, space="PSUM") as qpsp:
                    for ctg in range(4):          # groups of 4 col-tiles
                        q_ps = [qpsp.tile([128, QR], dt.float32, tag=f"q{s}",
                                          name=f"q_ps{s}") for s in range(4)]
                        for e in range(EC):
                            if ctg == 0:
                                nc.sync.dma_start(xq[:, e, :], xTqv[:, e, :])
                            elif e % 3 == ctg - 1:
                                nc.sync.dma_start(xk[:, e, :], xTv[:, e, :])
                            slab = wqp.tile([128, 512], dt.float32r, tag="wq")
                            nc.sync.dma_start(
                                slab[:],
                                Wq[bass.ts(e, 128), bass.ts(ctg, 512)])
                            for s in range(4):
                                nc.tensor.matmul(
                                    q_ps[s][:], slab[:, bass.ts(s, 128)],
                                    xq[:, e, :],
                                    start=(e == 0), stop=(e == EC - 1))

                        for s in range(4):
                            ct = ctg * 4 + s      # col tile = heads 2ct, 2ct+1
                            for half in range(2):
                                h = 2 * ct + half
                                pr = 64 * ((h % 8) // 4)
                                col = (h // 8) * 4 + (h % 8) % 4
                                nc.scalar.activation(
                                    qT[pr:pr + 64, col, :],
                                    q_ps[s][bass.ts(half, 64), :],
                                    AF.Identity,
                                    bias=bq_sb[bass.ts(half, 64), ct:ct + 1])

                # ---- k projection (transposed): kT[dk, g, key] ----
                with tc.tile_pool(name="wk", bufs=16) as wkp, \
                     tc.tile_pool(name="kps", bufs=3, space="PSUM") as kpsp:
                    for ct in range(4):           # col tiles over G*DK=512
                        k_ps = [kpsp.tile([128, 512], dt.float32, tag=f"k{i}",
                                          name=f"k_ps{i}") for i in range(2)]
                        for e in range(EC):
                            wt = wkp.tile([128, 128], dt.float32r, tag="wk")
                            nc.sync.dma_start(
                                wt[:], Wkv[bass.ts(e, 128), bass.ts(ct, 128)])
                            for half in range(2):
                                nc.tensor.matmul(
                                    k_ps[half][:], wt[:],
                                    xk[:, e, bass.ts(half, 512)],
                                    start=(e == 0), stop=(e == EC - 1))
                        for half in range(2):
                            for sub in range(2):
                                g = 2 * ct + sub
                                pr = 64 * (g // 4)
                                nc.scalar.activation(
                                    kT[pr:pr + 64, g % 4, bass.ts(half, 512)],
                                    k_ps[half][bass.ts(sub, 64), :],
                                    AF.Identity,
                                    bias=bk_sb[bass.ts(sub, 64), ct:ct + 1])

                # ---- v projection (natural): v[key, g, dk] + ones col ----
                # 8 psum accumulators (one per key tile); Wv streamed once
                with tc.tile_pool(name="wv", bufs=6) as wvp, \
                     tc.tile_pool(name="vps", bufs=1, space="PSUM") as vpsp:
                    v_ps = [vpsp.tile([128, 512], dt.float32, tag=f"v{kt}",
                                      name=f"v_ps{kt}") for kt in range(NKC)]
                    for e in range(EC):
                        wvt = wvp.tile([128, 512], dt.float32r, tag="wv")
                        nc.sync.dma_start(
                            wvt[:],
                            Wkv[bass.ts(e, 128), G * DK:2 * G * DK])
                        for kt in range(NKC):
                            nc.tensor.matmul(
                                v_ps[kt][:], xk[:, e, bass.ts(kt, 128)],
                                wvt[:], start=(e == 0), stop=False)
                    for kt in range(NKC):
                        # + bias row (K=1): ones[1,128].T @ bv[1,512]
                        nc.tensor.matmul(v_ps[kt][:], ones_r[:], bv_sb[:],
                                         start=False, stop=True)
                        ceng = nc.scalar if kt % 2 == 0 else nc.vector
                        if kt % 2 == 0:
                            nc.scalar.copy(vsb[:, kt, :, 0:DK],
                                           v_ps[kt][:].rearrange(
                                               "p (g d) -> p g d", g=G))
                        else:
                            nc.vector.tensor_copy(
                                vsb[:, kt, :, 0:DK],
                                v_ps[kt][:].rearrange("p (g d) -> p g d", g=G))
                        for g in range(G):
                            nc.gpsimd.memset(
                                vsb[:, kt, g, DK:DK + 1].bitcast(dt.float32), 1.0)


            # ================= phases 2+3 =================
            with tc.tile_pool(name="yT", bufs=1) as ytp:
                yT = ytp.tile([128, EC, QR], dt.float32r)     # 4 MB
                msk = ytp.tile([128, 8, 256], dt.float32r)    # 1 MB
                nc.sync.dma_start(msk[:], masks.rearrange("m p q -> p m q"))
                with tc.tile_pool(name="ex", bufs=6) as exp_, \
                     tc.tile_pool(name="bcst", bufs=4) as bcp, \
                     tc.tile_pool(name="wc", bufs=16) as wcp:
                    # ---- attention ----
                    with tc.tile_pool(name="scps", bufs=2, space="PSUM") as scp, \
                         tc.tile_pool(name="avps", bufs=2, space="PSUM") as avp, \
                         tc.tile_pool(name="opsw", bufs=1,
                                      space="PSUM") as opw:
                        # c_proj wave: oc=0, qt 0-1 accumulate during
                        # attention as yT e-chunks complete
                        o_w = [opw.tile([128, 512], dt.float32, tag=f"ow{qt}",
                                        name=f"o_w{qt}") for qt in range(2)]
                        w_started = [False, False]
                        for g in range(G):
                            for hp in range(2):
                                # qc1 (ACT-heavy) first, then qc0 (DVE-heavy):
                                # interleaving balances both engines
                                for qc in (1, 0):
                                    nkc = KCS[qc]
                                    h0 = g + 16 * hp  # heads h0, h0+8 (grp g)
                                    av = avp.tile([65, 512], dt.float32,
                                                  tag="av")
                                    base = 64 * (g // 4)
                                    rhs = qT[base:base + 64, :, :] \
                                        .rearrange("p (j c) q -> p j c q",
                                                   j=4)[:, 2 * hp:2 * hp + 2,
                                                        g % 4, bass.ts(qc, 256)]
                                    exs = []
                                    for kp2 in range(nkc // 2):
                                        # two key chunks share one psum/exp op
                                        sc2 = scp.tile([128, 2, 512],
                                                       dt.float32, tag="sc",
                                                       name="sc2")
                                        for j in range(2):
                                            kci = 2 * kp2 + j
                                            lhsT = kT[base:base + 64, g % 4,
                                                      bass.ts(kci, 128)]
                                            nc.tensor.matmul(
                                                sc2[:, j, :], lhsT, rhs,
                                                start=True, stop=True)
                                        ex2 = exp_.tile([128, 2, 2, 256],
                                                        dt.float32r, tag="ex",
                                                        name=f"ex{kp2}",
                                                        bufs=8)
                                        nc.scalar.activation(
                                            ex2[:].rearrange(
                                                "p k h q -> p (k h q)"),
                                            sc2[:].rearrange(
                                                "p k n -> p (k n)"),
                                            AF.Exp,
                                            scale=float(1.0 / np.sqrt(DK)))
                                        masked = (qc == 0) or 2 * kp2 >= 4
                                        if masked:
                                            mb = msk[:, 2 * kp2:2 * kp2 + 2,
                                                     None, :] \
                                                .broadcast_to((128, 2, 2, 256))
                                            nc.vector.tensor_mul(ex2[:],
                                                                 ex2[:], mb)
                                        exs.append(ex2)
                                    for kci in range(nkc):
                                        ex2 = exs[kci // 2]
                                        nc.tensor.matmul(
                                            av[:], vsb[:, kci, g, :],
                                            ex2[:, kci % 2, :, :].rearrange(
                                                "p h q -> p (h q)"),
                                            start=(kci == 0),
                                            stop=(kci == nkc - 1))
                                    # normalize: yT_h = av[0:64] * (1/av[64])
                                    rc = bcp.tile([1, 512], dt.float32,
                                                  tag="rc")
                                    nc.vector.reciprocal(rc[:], av[64:65, :])
                                    bcst = bcp.tile([64, 512], dt.float32,
                                                    tag="bcst")
                                    nc.gpsimd.partition_broadcast(bcst[:],
                                                                  rc[:])
                                    p0 = 64 * (g % 2)
                                    yTv = yT[p0:p0 + 64, :, :].rearrange(
                                        "p (j c) q -> p j c q", j=4)
                                    nc.vector.tensor_mul(
                                        yTv[:, 2 * hp:2 * hp + 2, g // 2,
                                            bass.ts(qc, 256)],
                                        av[0:64, :].rearrange(
                                            "p (h q) -> p h q", h=2),
                                        bcst[:, :].rearrange(
                                            "p (h q) -> p h q", h=2))
                                # after both qc of an odd g: two e-chunks of
                                # yT are complete; feed the c_proj wave
                                if g % 2 == 1:
                                    ready = (8 * hp + (g - 1) // 2,
                                             8 * hp + 4 + (g - 1) // 2)
                                    for e in ready:
                                        wsl = wcp.tile([128, 512],
                                                       dt.float32r, tag="wc",
                                                       name="wsl")
                                        nc.sync.dma_start(
                                            wsl[:],
                                            Wc[bass.ts(e, 128), 0:512])
                                        for qt in range(2):
                                            nc.tensor.matmul(
                                                o_w[qt][:],
                                                yT[:, e, bass.ts(qt, 128)],
                                                wsl[:],
                                                start=not w_started[qt],
                                                stop=False)
                                            w_started[qt] = True

                        # finish the wave: bias + copy + store
                        for qt in range(2):
                            nc.tensor.matmul(o_w[qt][:], ones_r[:],
                                             bc_sb[:, 0:512],
                                             start=False, stop=True)
                            otw = exp_.tile([128, 512], dt.float32,
                                            tag="ot")
                            nc.vector.tensor_copy(otw[:], o_w[qt][:])
                            nc.sync.dma_start(
                                y_out[bass.ts(qt, 128), 0:512], otw[:])

                    # ---- kv outputs (independent; emitted late) ----
                    for g in range(G):
                        pr = 64 * (g // 4)
                        nc.sync.dma_start(
                            kT_out[bass.ts(g, 64), :],
                            kT[pr:pr + 64, g % 4, :].bitcast(dt.float32))
                    for kt in range(NKC):
                        nc.sync.dma_start(
                            v_out[bass.ts(kt, 128), :].rearrange(
                                "p (g d) -> p g d", g=G),
                            vsb[:, kt, :, 0:DK].bitcast(dt.float32))

                    # ---- c_proj: y_out[qr, oc] = yT.T @ Wc + bc ----
                    with tc.tile_pool(name="ops", bufs=2,
                                      space="PSUM") as opp:
                        for oc in range(4):
                            o_ps = [opp.tile([128, 512], dt.float32, tag=f"o{qt}",
                                             name=f"o_ps{qt}")
                                    for qt in range(4)]
                            for e in range(EC):
                                slab = wcp.tile([128, 512], dt.float32r,
                                                tag="wc")
                                nc.sync.dma_start(
                                    slab[:],
                                    Wc[bass.ts(e, 128), bass.ts(oc, 512)])
                                for qt in range(4):
                                    if oc == 0 and qt < 2:
                                        continue
                                    nc.tensor.matmul(
                                        o_ps[qt][:],
                                        yT[:, e, bass.ts(qt, 128)],
                                        slab[:], start=(e == 0), stop=False)
                            for qt in range(4):
                                if oc == 0 and qt < 2:
                                    continue
                                nc.tensor.matmul(o_ps[qt][:], ones_r[:],
                                                 bc_sb[:, bass.ts(oc, 512)],
                                                 start=False, stop=True)
                                ot = exp_.tile([128, 512], dt.float32,
                                               tag="ot")
                                nc.scalar.copy(ot[:], o_ps[qt][:])
                                nc.sync.dma_start(
                                    y_out[bass.ts(qt, 128),
                                          bass.ts(oc, 512)],
                                    ot[:])

    nc.compile()
    return nc


_NC = None


def _get_nc():
    global _NC
    if _NC is None:
        _NC = build()
    return _NC


def make_in_maps(x, Wq, bq, Wkv, bkv, Wc, bc):
    x = np.asarray(x, np.float32)
    shared = {
        "Wq": np.asarray(Wq, np.float32),
        "Wkv": np.asarray(Wkv, np.float32),
        "Wc": np.asarray(Wc, np.float32),
        "bq": np.asarray(bq, np.float32).reshape(EC, 128),
        "bk": np.asarray(bkv, np.float32)[:G * DK].reshape(G * DK // 128, 128),
        "bv": np.asarray(bkv, np.float32)[G * DK:].reshape(1, G * DK),
        "bc": np.asarray(bc, np.float32).reshape(1, E),
    }
    masks_t = [_masks(0), _masks(1)]
    qrows_t = [_qrows(0), _qrows(1)]
    in_maps = []
    for c in range(N_CORES):
        b, t = c // 2, c % 2
        xTb = np.ascontiguousarray(x[b].T)
        m = dict(shared)
        m["xT"] = xTb
        m["xTq"] = np.ascontiguousarray(xTb[:, qrows_t[t]])
        m["masks"] = masks_t[t]
        in_maps.append(m)
    return in_maps


def assemble(results):
    """results: list of 8 per-core dicts -> (y, k, v) full outputs."""
    qrows_t = [_qrows(0), _qrows(1)]
    y = np.empty((B, T, E), np.float32)
    k = np.empty((B, G, T, DK), np.float32)
    v = np.empty((B, G, T, DK), np.float32)
    for c in range(N_CORES):
        b, t = c // 2, c % 2
        y[b, qrows_t[t]] = results[c]["y_out"]
        if t == 0:
            k[b] = results[c]["kT_out"].reshape(G, DK, T).transpose(0, 2, 1)
            v[b] = results[c]["v_out"].reshape(T, G, DK).transpose(1, 0, 2)
    k_full = np.tile(k, (1, REP, 1, 1))
    v_full = np.tile(v, (1, REP, 1, 1))
    return y, k_full, v_full


def kernel(**inputs):
    nc = _get_nc()
    in_maps = make_in_maps(**inputs)
    try:
        res = bass_utils.run_bass_kernel_spmd(nc, in_maps,
                                              core_ids=list(range(N_CORES)))
    except Exception:
        # transient device errors (e.g. NRT_EXEC_UNIT_UNRECOVERABLE) have
        # been observed to clear on retry
        res = bass_utils.run_bass_kernel_spmd(nc, in_maps,
                                              core_ids=list(range(N_CORES)))
    return assemble(res.results)
